# revision 13
# baseline (speedup 1.0000x reference)
"""Trainium2 Bass kernel for DigitCapsuleLayer (single routing iteration).

Math: with num_iterations == 1 the routing coefficients are uniform 1/R, so

    v[b,c,o] = squash( (1/R) * sum_{r,i} x[b,r,i] * W[0,r,c,o,i] )

one [B=128, K=32768] x [K=32768, N=1024] matmul + a tiny squash.  W is read
exactly once -> HBM-bound.

Sharding (8 cores): split the OUTPUT capsule dim C=32 so each core owns 4
capsules (128 columns) and computes them completely locally from its 8 MB
W slice + the full x (8 MB) -- both bf16 (tolerance 2e-2, bf16 lands at
~2e-3; the 1/R coefficient is folded into W host-side, an exact exponent
shift).  No collective: profiling the original K-sharded + AllToAll version
showed the ncfw machinery (entry-barrier rank skew + ~11 us setup + a 30 us
AllToAll for 256 KB) burning ~73 us after a 63 us near-roofline stream, and
any cross-core dependency also imports rank start-skew into the measured
span.  Trading 7 MB of replicated x for zero collectives wins by ~2.4x.

x and W are interleaved per 128-row contraction chunk in ONE packed DRAM
tensor [128 part, 256 kc, 128 x-cols | 128 w-cols] so the stream is a
single FIFO of large fully-contiguous DMAs (up to 20 KB/partition lines);
group sizes ramp up then shrink at the end because a group's matmuls can
only start once the whole group lands (a 52-chunk tail group cost 5.7 us of
post-stream drain in an earlier rev; the 4-chunk tail costs ~0.5 us).
Measured: the stream runs at the HBM-per-core limit (~358 GB/s fair-share,
up to ~400 GB/s when the stack-pair neighbor lags).

Raw bass (no TileContext), hand-scheduled with explicit semaphores: the
Tile framework's ~7 us preamble (entry barriers, ordering modes, pool
memsets) and ~3 us teardown drains shrink to ~1.5 us total -- the 10 stream
DMAs are the sync engine's first instructions.  Every RAW edge crosses or
follows an engine pipeline whose writes land asynchronously, so each gets
an explicit sem hop (what the Tile scheduler normally automates):

  sync:   dma g0..g9 (each then_inc sem_dma[g] 16) .. wait(sem_dve); out-DMA
  tensor: [wait sem_dma[g]>=16; matmuls of group g] x 10; last inc sem_pe
  scalar: warm sqrt (loads the ACT table off the critical path);
          wait(sem_pe); square(PSUM->s2); later sqrt(sq)->rt
  vector: reduce s2 -> sq; den=sq+1; rec=1/den; fac=rt*rec; v=ps*fac (bf16)
"""

import numpy as np
import ml_dtypes

import concourse.bacc as bacc
import concourse.bass as bass
import concourse.bass_utils as bass_utils
import concourse.mybir as mybir

B, R, C, I, O = 128, 2048, 32, 16, 32
NCORES = 8
CSH = C // NCORES
NCOL = CSH * O
KC = (R * I) // 128
FREE = B + NCOL
GROUPS = [24, 40, 40, 40, 40, 40, 24, 6, 2]
assert sum(GROUPS) == KC

BF16 = ml_dtypes.bfloat16


def _build_program():
    nc = bacc.Bacc(
        "TRN2", target_bir_lowering=False, debug=False, num_devices=NCORES
    )
    f32 = mybir.dt.float32
    bf16 = mybir.dt.bfloat16

    xw = nc.dram_tensor("xw", [128, KC, FREE], bf16, kind="ExternalInput").ap()
    out = nc.dram_tensor("out", [B, NCOL], bf16, kind="ExternalOutput").ap()

    sem_dma = [nc.alloc_semaphore(f"sem_dma{i}") for i in range(len(GROUPS))]
    sem_pe = nc.alloc_semaphore("sem_pe")
    sem_act = nc.alloc_semaphore("sem_act")
    sem_dve = nc.alloc_semaphore("sem_dve")

    with (
        nc.sbuf_tensor("sb", [128, KC, FREE], bf16) as sb_h,
        nc.sbuf_tensor("s2", [128, NCOL], f32) as s2_h,
        nc.sbuf_tensor("sq", [128, CSH], f32) as sq_h,
        nc.sbuf_tensor("rt", [128, CSH], f32) as rt_h,
        nc.sbuf_tensor("den", [128, CSH], f32) as den_h,
        nc.sbuf_tensor("rec", [128, CSH], f32) as rec_h,
        nc.sbuf_tensor("fac", [128, CSH], f32) as fac_h,
        nc.sbuf_tensor("v", [128, NCOL], bf16) as v_h,
        nc.sbuf_tensor("warm", [1, 1], f32) as warm_h,
        nc.psum_tensor("ps", [B, NCOL], f32) as ps_h,
    ):
        sb, s2, sq = sb_h.ap(), s2_h.ap(), sq_h.ap()
        rt, den, rec, fac, v, warm = (
            rt_h.ap(), den_h.ap(), rec_h.ap(), fac_h.ap(), v_h.ap(), warm_h.ap()
        )
        ps = ps_h.ap()

        # Stream DMAs: very first instructions on the sync queue.
        g0 = 0
        for gi, gsz in enumerate(GROUPS):
            nc.sync.dma_start(
                sb[:, g0 : g0 + gsz, :], xw[:, g0 : g0 + gsz, :]
            ).then_inc(sem_dma[gi], 16)
            g0 += gsz

        # Matmul chain, paced by the DMA semaphore at group granularity.
        g0 = 0
        last_mm = None
        for gi, gsz in enumerate(GROUPS):
            nc.tensor.wait_ge(sem_dma[gi], 16)
            for kc in range(g0, g0 + gsz):
                last_mm = nc.tensor.matmul(
                    ps,
                    sb[:, kc, 0:B],
                    sb[:, kc, B:FREE],
                    start=(kc == 0),
                    stop=(kc == KC - 1),
                )
            g0 += gsz
        last_mm.then_inc(sem_pe, 1)

        # ACT: warm the Sqrt table way before it's needed, then the
        # square+accum (sum over o per capsule) and sqrt.
        sem_w = nc.alloc_semaphore("sem_w")
        nc.gpsimd.memset(warm, 0.0).then_inc(sem_w, 1)
        nc.scalar.wait_ge(sem_w, 1)
        nc.scalar.sqrt(warm, warm)
        # Squash pipelined in two capsule-halves (columns 0:64 and 64:128)
        # so the first half's output DMA fires while the second half is
        # still on the vector engine: plain square (beats accum_out --
        # ACTIVATION_READ_ACCUMULATOR costs ~290 ns per group), DVE
        # reduce, ACT sqrt, DVE finish.  Half-chains are interleaved on
        # DVE so each RAW sem hop hides under the other half's op.
        HC = CSH // 2           # 2 capsules per half
        HN = NCOL // 2          # 64 columns per half
        sem_sq = nc.alloc_semaphore("sem_sq")
        sem_rq = nc.alloc_semaphore("sem_rq")
        sem_v = nc.alloc_semaphore("sem_v")
        sem_out = nc.alloc_semaphore("sem_out")

        def half(t, h):
            return t[:, h * HN : (h + 1) * HN]

        def halfc(t, h):
            return t[:, h * HC : (h + 1) * HC]

        nc.scalar.wait_ge(sem_pe, 1)
        for h in range(2):
            nc.scalar.square(half(s2, h), half(ps, h)).then_inc(sem_sq, 1)
        for h in range(2):
            nc.vector.wait_ge(sem_sq, h + 1)
            nc.vector.reduce_sum(
                halfc(sq, h),
                half(s2, h).rearrange("p (cl o) -> p cl o", o=O),
                axis=mybir.AxisListType.X,
            ).then_inc(sem_rq, 1)
        for h in range(2):
            nc.scalar.wait_ge(sem_rq, h + 1)
            nc.scalar.sqrt(halfc(rt, h), halfc(sq, h)).then_inc(sem_act, 1)
        # DVE: den = sq+1; rec = 1/den; fac = rt*rec; v = s*fac, halves
        # interleaved (sem_v counts: den0,den1,rec0,rec1,fac0,fac1).
        for h in range(2):
            nc.vector.wait_ge(sem_rq, h + 1)
            nc.vector.tensor_scalar_add(
                halfc(den, h), halfc(sq, h), 1.0
            ).then_inc(sem_v, 1)
        for h in range(2):
            nc.vector.wait_ge(sem_v, h + 1)
            nc.vector.reciprocal(halfc(rec, h), halfc(den, h)).then_inc(sem_v, 1)
        for h in range(2):
            nc.vector.wait_ge(sem_act, h + 1)
            nc.vector.wait_ge(sem_v, h + 3)
            nc.vector.tensor_mul(
                out=halfc(fac, h), in0=halfc(rt, h), in1=halfc(rec, h)
            ).then_inc(sem_v, 1)
        for h in range(2):
            nc.vector.wait_ge(sem_v, h + 5)
            nc.vector.tensor_tensor(
                half(v, h).rearrange("p (cl o) -> p cl o", o=O),
                half(ps, h).rearrange("p (cl o) -> p cl o", o=O),
                halfc(fac, h)[:, :, None].to_broadcast((128, HC, O)),
                mybir.AluOpType.mult,
            ).then_inc(sem_dve, 1)
        # Two output DMAs on separate HWDGE rings (sync + the now-idle
        # scalar ring) so their completion receipts overlap; the first
        # also hides under the second half's DVE work.  Every raw
        # dma_start needs a completion semaphore (HWDGE codegen asserts
        # otherwise).
        nc.sync.wait_ge(sem_dve, 1)
        nc.sync.dma_start(half(out, 0), half(v, 0)).then_inc(sem_out, 16)
        nc.scalar.wait_ge(sem_dve, 2)
        nc.scalar.dma_start(half(out, 1), half(v, 1)).then_inc(sem_out, 16)
        nc.sync.wait_ge(sem_out, 32)

    nc.compile()
    return nc


# Host-side packing identical to kernel.py v3.
def _pack_inputs(x: np.ndarray, W: np.ndarray):
    xb = x.astype(BF16)
    x_prep = np.ascontiguousarray(
        xb.reshape(B, KC, 8, I).transpose(2, 3, 1, 0)
    ).reshape(128, KC, B)
    Wb = (W[0] * (1.0 / R)).astype(BF16)
    in_maps = []
    for m in range(NCORES):
        Wm = Wb[:, m * CSH : (m + 1) * CSH]
        Wm = Wm.reshape(KC, 8, CSH, O, I)
        w_prep = np.ascontiguousarray(
            Wm.transpose(1, 4, 0, 2, 3)
        ).reshape(128, KC, NCOL)
        in_maps.append({"xw": np.concatenate([x_prep, w_prep], axis=2)})
    return in_maps


_CACHED_NC = None


def _get_nc():
    global _CACHED_NC
    if _CACHED_NC is None:
        _CACHED_NC = _build_program()
    return _CACHED_NC


def kernel(x: np.ndarray, W: np.ndarray, _trace: bool = False):
    x = np.ascontiguousarray(np.asarray(x, dtype=np.float32))
    W = np.ascontiguousarray(np.asarray(W, dtype=np.float32))
    nc = _get_nc()
    in_maps = _pack_inputs(x, W)
    res = bass_utils.run_bass_kernel_spmd(
        nc, in_maps, core_ids=list(range(NCORES)), trace=_trace
    )
    out = np.concatenate(
        [np.asarray(res.results[m]["out"], dtype=np.float32).reshape(B, CSH, O)
         for m in range(NCORES)],
        axis=1,
    ).reshape(B, C, O, 1)
    if _trace:
        return out, res
    return out
